# revision 19
# baseline (speedup 1.0000x reference)
"""ARCformer block on 8 TRN2 NeuronCores.

Sharding: data-parallel over batch (4) x 2-way split of the sequence
positions within each batch pair. Each core handles 640 of the 1280
"needed" query positions of one batch (512 x-positions + 128 state
positions) and computes K/V for 640 of the 1280 unique tokens; K/V
shards are exchanged with the pair partner via 4 chunked pair
AllGathers (one per 4-head group) that overlap the Q projection and
attention.

On-core layout: activations are kept feature-major (feature on the
SBUF partition axis, tokens on the free axis), so every linear layer
is lhsT=W-chunk @ rhs=X^T-chunk with no transposes. Attention runs in
the "scores transposed" orientation [keys, queries]: softmax
numerators via ACT exp (max-free; |scores| <= ~8 for this problem),
causal mask applied as a 0/1 multiply, denominators via an all-ones
matmul accumulated in PSUM alongside P@V, normalization folded into
the PSUM drain. LayerNorm stats are computed with ones-matmuls
(pre-broadcast across partitions); rstd = exp(-0.5*ln(var+eps)) keeps
ACT in the natural_log_exp table set. All GEMMs run in bf16 (weights
pre-cast on the host) with fp32 PSUM accumulation; LayerNorm sums and
the residual chain stay in fp32/f32r.
"""

import numpy as np
import ml_dtypes

import concourse.bass as bass
import concourse.tile as tile
from concourse import bacc, mybir
from concourse.bass_utils import run_bass_kernel_spmd
from concourse.masks import make_identity

F32 = mybir.dt.float32
F32R = mybir.dt.float32r
FP8 = mybir.dt.float8e4
BF16 = mybir.dt.bfloat16
AF = mybir.ActivationFunctionType
OP = mybir.AluOpType

B, L, S, D = 4, 1024, 256, 2048
H, DK, DV = 16, 128, 128
DH = 8192
T = S + L + S  # 1536
P = 128
DC = D // P  # 16
NQ = 640  # per-core query positions (512 x-part + 128 state-part)
NXP = 512  # x-part columns
NKV = 640  # per-core unique kv tokens
KCN = 12  # key chunks (1536/128)
EPS = 1e-5
ISQ = float(1.0 / np.sqrt(DK))

# attention spans: (q0, qn, klim, mlim)
SPANS = [(0, 512, 10, 2), (512, 128, 12, 10)]
# projection token spans
PSPANS = [(0, 512), (512, 128)]

# vecs columns
VC_BQ, VC_BK, VC_BM1, VC_BM2, VC_CB = 0, 16, 32, 96, 112
VC_G1, VC_B1, VC_G2, VC_B2 = 128, 144, 160, 176
VC_S2 = 192
VC_S1 = 208
VC_N = 272

_NC = None
LAST_RESULT = None
DEBUG = False


def _build():
    nc = bacc.Bacc("TRN2", target_bir_lowering=False, debug=False, num_devices=8)

    def inp(name, shape, dt):
        return nc.dram_tensor(name, shape, dt, kind="ExternalInput").ap()

    xq = inp("xq", [P, DC, NQ], BF16)
    xres = inp("xres", [P, DC, NXP], F32)
    kvt = inp("kvt", [P, DC, NKV], BF16)
    maskt = inp("maskt", [P, KCN, NQ], BF16)
    wq = inp("wq", [D, D], BF16)
    wk = inp("wk", [D, D], BF16)
    wv = inp("wv", [D, D], BF16)
    wo1 = inp("wo1", [D, D], BF16)
    wo2 = inp("wo2", [D, D], BF16)
    wm1 = inp("wm1", [D, DH], FP8)
    wm2 = inp("wm2", [DH, D], FP8)
    vecs = inp("vecs", [P, VC_N], F32)
    out = nc.dram_tensor("out", [P, DC, NQ], F32, kind="ExternalOutput").ap()
    dbg = {}
    if DEBUG:
        dbg["dq"] = nc.dram_tensor("dq", [P, DC, NQ], BF16, kind="ExternalOutput").ap()
        dbg["dkt"] = nc.dram_tensor("dkt", [P, T], BF16, kind="ExternalOutput").ap()
        dbg["dvt"] = nc.dram_tensor("dvt", [P, KCN, P], BF16, kind="ExternalOutput").ap()
        dbg["det"] = nc.dram_tensor("det", [P, KCN, NQ], BF16, kind="ExternalOutput").ap()
        dbg["drt"] = nc.dram_tensor("drt", [P, 2, 512], F32, kind="ExternalOutput").ap()
        dbg["do"] = nc.dram_tensor("do", [P, DC, NQ], BF16, kind="ExternalOutput").ap()
        dbg["dt"] = nc.dram_tensor("dt", [P, DC, NXP], F32R, kind="ExternalOutput").ap()
        dbg["dh"] = nc.dram_tensor("dh", [P, DC, NXP], BF16, kind="ExternalOutput").ap()
        dbg["dt2"] = nc.dram_tensor("dt2", [P, DC, NXP], F32R, kind="ExternalOutput").ap()

    with tile.TileContext(nc, pool_alloc_mode="queue") as tc:
        cm_const = tc.tile_pool(name="const", bufs=1)
        const = cm_const.__enter__()
        vecs_sb = const.tile([P, VC_N], F32)
        nc.sync.dma_start(vecs_sb[:], vecs)
        ones_f = const.tile([P, P], F32)
        nc.vector.memset(ones_f[:], 1.0)
        ones_r = const.tile([P, P], F32R)
        nc.vector.tensor_copy(ones_r[:], ones_f[:])
        ones_bf = const.tile([P, P], BF16)
        nc.vector.memset(ones_bf[:], 1.0)
        eps_sb = const.tile([P, 1], F32)
        nc.vector.memset(eps_sb[:], EPS)
        ident_bf = const.tile([P, P], BF16)
        make_identity(nc, ident_bf[:])

        cm_dram = tc.tile_pool(name="dram", bufs=1, space="DRAM")
        dram = cm_dram.__enter__()
        kvb_in = [
            dram.tile([2, 4 * P, NKV], BF16, name=f"kvbin{g}") for g in range(4)
        ]
        kvb_out = [
            dram.tile([2, 2, 4 * P, NKV], BF16, name=f"kvbout{g}") for g in range(4)
        ]

        cm_qm = tc.tile_pool(name="p_qm", bufs=1)
        p_qm = cm_qm.__enter__()
        q_sb = p_qm.tile([P, DC, NQ], BF16, name="q_sb")
        mask_sb = p_qm.tile([P, KCN, NQ], BF16, name="mask_sb")

        # ---------------- phase 1: QKV projections + pair AllGather ----
        # 4 f-chunks per weight tile [128, 512]; PSUM [128, 2, 512] per f.
        def proj(w_ap, src_sb, fgs, drain, psp, wpool, ncols):
            for fg in fgs:
                pss = [
                    psp.tile([P, 2, 512], F32, tag="pp", name=f"pp{fg}_{j}")
                    for j in range(4)
                ]
                for d in range(DC):
                    wt = wpool.tile([P, 512], BF16, tag="w", name="wt")
                    nc.sync.dma_start(
                        wt[:], w_ap[d * P : (d + 1) * P, fg * 512 : (fg + 1) * 512]
                    )
                    for j in range(4):
                        for s, (s0, sn) in enumerate(PSPANS):
                            if s0 >= ncols:
                                continue
                            nc.tensor.matmul(
                                pss[j][:, s, :sn],
                                lhsT=wt[:, j * P : (j + 1) * P],
                                rhs=src_sb[:, d, s0 : s0 + sn],
                                start=(d == 0),
                                stop=(d == DC - 1),
                            )
                for j in range(4):
                    drain(fg * 4 + j, pss[j])

        cm_in = tc.tile_pool(name="p_in", bufs=1)
        p_in = cm_in.__enter__()
        xq_sb = p_in.tile([P, DC, NQ], BF16, name="xq_sb")
        kv_sb = p_in.tile([P, DC, NKV], BF16, name="kv_sb")
        for d in range(DC):
            nc.sync.dma_start(kv_sb[:, d, :], kvt[:, d, :])
        for d in range(DC):
            nc.sync.dma_start(xq_sb[:, d, :], xq[:, d, :])
        nc.sync.dma_start(mask_sb[:], maskt)

        cm_w1 = tc.tile_pool(name="w1", bufs=10)
        wpool1 = cm_w1.__enter__()
        cm_st1 = tc.tile_pool(name="st1", bufs=4)
        stage1 = cm_st1.__enter__()
        cm_ps1 = tc.tile_pool(name="ps1", bufs=4, space="PSUM")
        psp1 = cm_ps1.__enter__()

        def drain_kv(which, f, ps):
            st = stage1.tile([P, NKV], BF16, tag="kvst", name="kvst")
            for s, (s0, sn) in enumerate(PSPANS):
                if which == 0:
                    nc.scalar.activation(
                        st[:, s0 : s0 + sn],
                        ps[:, s, :sn],
                        AF.Identity,
                        bias=vecs_sb[:, VC_BK + f : VC_BK + f + 1],
                    )
                else:
                    nc.scalar.activation(st[:, s0 : s0 + sn], ps[:, s, :sn], AF.Copy)
            g, r = f // 4, f % 4
            nc.sync.dma_start(kvb_in[g][which, r * P : (r + 1) * P, :], st[:])

        def drain_q(f, ps):
            for s, (s0, sn) in enumerate(PSPANS):
                nc.scalar.activation(
                    q_sb[:, f, s0 : s0 + sn],
                    ps[:, s, :sn],
                    AF.Identity,
                    bias=vecs_sb[:, VC_BQ + f : VC_BQ + f + 1],
                )

        for g in range(4):
            proj(wk, kv_sb, [g], lambda f, ps: drain_kv(0, f, ps), psp1, wpool1, NKV)
            proj(wv, kv_sb, [g], lambda f, ps: drain_kv(1, f, ps), psp1, wpool1, NKV)
            nc.gpsimd.collective_compute(
                "AllGather",
                OP.bypass,
                replica_groups=[[0, 1], [2, 3], [4, 5], [6, 7]],
                ins=[kvb_in[g].opt()],
                outs=[kvb_out[g].opt()],
            )
        proj(wq, xq_sb, list(range(4)), drain_q, psp1, wpool1, NQ)
        if DEBUG:
            nc.sync.dma_start(dbg["dq"][:], q_sb[:])

        cm_ps1.__exit__(None, None, None)
        cm_st1.__exit__(None, None, None)
        cm_w1.__exit__(None, None, None)
        cm_in.__exit__(None, None, None)

        # ---------------- phase 2: attention ---------------------------
        cm_o = tc.tile_pool(name="p_o", bufs=1, side="right")
        o_sb = cm_o.__enter__().tile([P, DC, NQ], BF16, name="o_sb")

        # pre-pass: transpose all V chunks to token-major [nk, DV]
        cm_va = tc.tile_pool(name="p_va", bufs=1)
        vt_all = cm_va.__enter__().tile([P, H, KCN, P], BF16, name="vt_all")
        cm_vp = tc.tile_pool(name="vp", bufs=4)
        vpool = cm_vp.__enter__()
        cm_apt = tc.tile_pool(name="apt", bufs=4, space="PSUM")
        aps_t = cm_apt.__enter__()
        for hh in range(H):
            g, hr = hh // 4, hh % 4
            vts = vpool.tile([P, T], BF16, tag="vts", name="vts")
            nc.sync.dma_start(
                vts[:, 0:640], kvb_out[g][0, 1, hr * P : (hr + 1) * P, :]
            )
            nc.sync.dma_start(
                vts[:, 640:1280], kvb_out[g][1, 1, hr * P : (hr + 1) * P, :]
            )
            nc.sync.dma_start(
                vts[:, 1280:1536], kvb_out[g][0, 1, hr * P : (hr + 1) * P, 0:256]
            )
            for kc in range(KCN):
                tp = aps_t.tile([P, P], BF16, tag="tp", name="tp")
                nc.tensor.transpose(
                    tp[:], vts[:, kc * P : (kc + 1) * P], ident_bf[:]
                )
                nc.vector.tensor_copy(vt_all[:, hh, kc, :], tp[:])
        cm_apt.__exit__(None, None, None)
        cm_vp.__exit__(None, None, None)

        cm_kp = tc.tile_pool(name="kp", bufs=2)
        kpool = cm_kp.__enter__()
        cm_ep = tc.tile_pool(name="ep", bufs=3)
        epool = cm_ep.__enter__()
        cm_rp = tc.tile_pool(name="rp", bufs=2)
        rpool = cm_rp.__enter__()
        cm_aps = tc.tile_pool(name="aps", bufs=2, space="PSUM")
        aps_s = cm_aps.__enter__()
        cm_apo = tc.tile_pool(name="apo", bufs=1, space="PSUM")
        aps_o = cm_apo.__enter__()
        cm_apu = tc.tile_pool(name="apu", bufs=1, space="PSUM")
        aps_u = cm_apu.__enter__()

        def attn_front(hh):
            g, hr = hh // 4, hh % 4
            kt = kpool.tile([P, T], BF16, tag="kt", name="kt")
            nc.sync.dma_start(kt[:, 0:640], kvb_out[g][0, 0, hr * P : (hr + 1) * P, :])
            nc.sync.dma_start(
                kt[:, 640:1280], kvb_out[g][1, 0, hr * P : (hr + 1) * P, :]
            )
            nc.sync.dma_start(
                kt[:, 1280:1536], kvb_out[g][0, 0, hr * P : (hr + 1) * P, 0:256]
            )
            et = epool.tile([P, KCN, NQ], BF16, tag="et", name="et")
            for s, (q0, qn, klim, mlim) in enumerate(SPANS):
                for kc in range(0, klim, 2):
                    pss = aps_s.tile([P, 2, 512], F32, tag="ps_s", name="ps_s")
                    for u in range(2):
                        nc.tensor.matmul(
                            pss[:, u, :qn],
                            lhsT=kt[:, (kc + u) * P : (kc + u + 1) * P],
                            rhs=q_sb[:, hh, q0 : q0 + qn],
                            start=True,
                            stop=True,
                        )
                    nc.scalar.activation(
                        et[:, kc : kc + 2, q0 : q0 + qn],
                        pss[:, :, :qn],
                        AF.Exp,
                        scale=ISQ,
                    )
                    if kc >= mlim:
                        nc.vector.tensor_mul(
                            et[:, kc : kc + 2, q0 : q0 + qn],
                            et[:, kc : kc + 2, q0 : q0 + qn],
                            mask_sb[:, kc : kc + 2, q0 : q0 + qn],
                        )
            return et

        def attn_back(hh, et):
            vt = vt_all[:, hh]
            ps_o = aps_o.tile([P, 2, 512], F32, tag="ps_o", name="ps_o")
            ps_u = aps_u.tile([P, 2, 512], F32, tag="ps_u", name="ps_u")
            for s, (q0, qn, klim, _) in enumerate(SPANS):
                for kc in range(klim):
                    nc.tensor.matmul(
                        ps_u[:, s, :qn],
                        lhsT=ones_bf[:],
                        rhs=et[:, kc, q0 : q0 + qn],
                        start=(kc == 0),
                        stop=(kc == klim - 1),
                    )
                for kc in range(klim):
                    nc.tensor.matmul(
                        ps_o[:, s, :qn],
                        lhsT=vt[:, kc, :],
                        rhs=et[:, kc, q0 : q0 + qn],
                        start=(kc == 0),
                        stop=(kc == klim - 1),
                    )
            rt = rpool.tile([P, 2, 512], F32, tag="rt", name="rt")
            for s, (q0, qn, _, _) in enumerate(SPANS):
                nc.vector.reciprocal_approx_fast(rt[:, s, :qn], ps_u[:, s, :qn])
                nc.vector.tensor_mul(
                    o_sb[:, hh, q0 : q0 + qn], ps_o[:, s, :qn], rt[:, s, :qn]
                )

        prev = None
        for hh in range(H):
            et = attn_front(hh)
            if prev is not None:
                attn_back(*prev)
            prev = (hh, et)
        attn_back(*prev)
        if DEBUG:
            nc.sync.dma_start(dbg["do"][:], o_sb[:])

        cm_apu.__exit__(None, None, None)
        cm_apo.__exit__(None, None, None)
        cm_aps.__exit__(None, None, None)
        cm_rp.__exit__(None, None, None)
        cm_ep.__exit__(None, None, None)
        cm_kp.__exit__(None, None, None)
        cm_va.__exit__(None, None, None)
        cm_qm.__exit__(None, None, None)

        # ---------------- phase 3: output projection -------------------
        cm_t = tc.tile_pool(name="p_t", bufs=1)
        t_sb = cm_t.__enter__().tile([P, DC, NXP], F32R, name="t_sb")
        cm_u1 = tc.tile_pool(name="p_u1", bufs=1)
        u1 = cm_u1.__enter__().tile([P, DC, NQ], BF16, name="u1")
        cm_w3 = tc.tile_pool(name="w3", bufs=10)
        wpool3 = cm_w3.__enter__()
        cm_st3 = tc.tile_pool(name="st3", bufs=4)
        stage3 = cm_st3.__enter__()
        cm_ps3 = tc.tile_pool(name="ps3", bufs=4, space="PSUM")
        psp3 = cm_ps3.__enter__()

        def drain_u1(f, ps):
            for s, (s0, sn) in enumerate(PSPANS):
                nc.scalar.activation(u1[:, f, s0 : s0 + sn], ps[:, s, :sn], AF.Copy)

        proj(wo1, o_sb, list(range(4)), drain_u1, psp3, wpool3, NQ)

        def drain_u2(f, ps):
            cb = vecs_sb[:, VC_CB + f : VC_CB + f + 1]
            xst = stage3.tile([P, NXP], F32, tag="xst", name="xst")
            nc.sync.dma_start(xst[:], xres[:, f, :])
            nc.vector.scalar_tensor_tensor(
                t_sb[:, f, :], ps[:, 0, :512], cb, xst[:], OP.add, OP.add
            )
            sst = stage3.tile([P, P], F32, tag="sst", name="sst")
            nc.vector.tensor_scalar_add(sst[:], ps[:, 1, :128], cb)
            nc.sync.dma_start(out[:, f, NXP:NQ], sst[:])

        proj(wo2, u1, list(range(4)), drain_u2, psp3, wpool3, NQ)
        if DEBUG:
            nc.sync.dma_start(dbg["dt"][:], t_sb[:])

        cm_ps3.__exit__(None, None, None)
        cm_st3.__exit__(None, None, None)
        cm_w3.__exit__(None, None, None)
        cm_u1.__exit__(None, None, None)
        cm_o.__exit__(None, None, None)

        # ---------------- layernorm helper -----------------------------
        def layernorm(src, emit):
            cm_sq = tc.tile_pool(name="lnsq", bufs=1)
            sq = cm_sq.__enter__().tile([P, DC, NXP], F32R, name="sq")
            cm_stat = tc.tile_pool(name="lnstat", bufs=1)
            statp = cm_stat.__enter__()
            cm_tmp = tc.tile_pool(name="lntmp", bufs=3)
            tmpp = cm_tmp.__enter__()
            cm_psl = tc.tile_pool(name="lnps", bufs=2, space="PSUM")
            pslp = cm_psl.__enter__()

            for f in range(DC):
                nc.vector.tensor_mul(sq[:, f, :], src[:, f, :], src[:, f, :])
            ps_m = pslp.tile([P, 512], F32, tag="lm", name="lnm")
            ps_v = pslp.tile([P, 512], F32, tag="lv", name="lnv")
            for d in range(DC):
                nc.tensor.matmul(
                    ps_m[:, :NXP], lhsT=ones_r[:], rhs=src[:, d, :],
                    start=(d == 0), stop=(d == DC - 1),
                )
            for d in range(DC):
                nc.tensor.matmul(
                    ps_v[:, :NXP], lhsT=ones_r[:], rhs=sq[:, d, :],
                    start=(d == 0), stop=(d == DC - 1),
                )
            mean = statp.tile([P, NXP], F32, name="mean")
            nc.vector.tensor_scalar_mul(mean[:], ps_m[:, :NXP], 1.0 / D)
            m2 = statp.tile([P, NXP], F32, name="m2")
            nc.vector.tensor_mul(m2[:], mean[:], mean[:])
            var = statp.tile([P, NXP], F32, name="var")
            nc.vector.scalar_tensor_tensor(
                var[:], ps_v[:, :NXP], 1.0 / D, m2[:], OP.mult, OP.subtract
            )
            lnv = statp.tile([P, NXP], F32, name="lnv2")
            nc.scalar.activation(lnv[:], var[:], AF.Ln, bias=eps_sb[:])
            rstd = statp.tile([P, NXP], F32, name="rstd")
            nc.scalar.activation(rstd[:], lnv[:], AF.Exp, scale=-0.5)
            for f in range(DC):
                tmp = tmpp.tile([P, NXP], F32, tag="lnt", name="lnt")
                nc.vector.tensor_sub(tmp[:], src[:, f, :], mean[:])
                nc.vector.tensor_mul(tmp[:], tmp[:], rstd[:])
                emit(f, tmp)

            cm_psl.__exit__(None, None, None)
            cm_tmp.__exit__(None, None, None)
            cm_stat.__exit__(None, None, None)
            cm_sq.__exit__(None, None, None)

        # ---------------- phase 4: LN1 ---------------------------------
        cm_h = tc.tile_pool(name="p_h", bufs=1, side="right")
        h_sb = cm_h.__enter__().tile([P, DC, NXP], BF16, name="h_sb")

        def emit_h(f, tmp):
            nc.scalar.activation(
                h_sb[:, f, :], tmp[:], AF.Identity,
                bias=vecs_sb[:, VC_B1 + f : VC_B1 + f + 1],
                scale=vecs_sb[:, VC_G1 + f : VC_G1 + f + 1],
            )

        layernorm(t_sb, emit_h)
        if DEBUG:
            nc.sync.dma_start(dbg["dh"][:], h_sb[:])
        cm_t.__exit__(None, None, None)
        cm_h8 = tc.tile_pool(name="p_h8", bufs=1)
        h8 = cm_h8.__enter__().tile([P, DC, NXP], FP8, name="h8")
        for f in range(DC):
            nc.vector.tensor_copy(h8[:, f, :], h_sb[:, f, :])

        # ---------------- phase 5: MLP layer 1 -------------------------
        cm_m1 = tc.tile_pool(name="p_m1", bufs=1, side="right")
        m1 = cm_m1.__enter__().tile([P, DH // P, NXP], FP8, name="m1")
        cm_w5 = tc.tile_pool(name="w5", bufs=6)
        wpool5 = cm_w5.__enter__()
        cm_ps5 = tc.tile_pool(name="ps5", bufs=8, space="PSUM")
        psp5 = cm_ps5.__enter__()
        KD1 = DC // 2  # 8 double-row contraction steps
        for fg in range(8):
            pss = [
                psp5.tile([P, 512], F32, tag="mp", name=f"mp{fg}_{j}")
                for j in range(8)
            ]
            for k2 in range(KD1):
                wt = wpool5.tile([P, 2, 1024], FP8, tag="wm1", name="wm1t")
                nc.sync.dma_start(
                    wt[:],
                    wm1[
                        k2 * 2 * P : (k2 + 1) * 2 * P,
                        fg * 1024 : (fg + 1) * 1024,
                    ].rearrange("(h p) c -> p h c", p=P),
                )
                for j in range(8):
                    nc.tensor.matmul(
                        pss[j][:],
                        lhsT=wt[:, :, j * P : (j + 1) * P],
                        rhs=h8[:, 2 * k2 : 2 * k2 + 2, :],
                        start=(k2 == 0),
                        stop=(k2 == KD1 - 1),
                        perf_mode=mybir.MatmulPerfMode.DoubleRow,
                    )
            for j in range(8):
                f = fg * 8 + j
                nc.scalar.activation(
                    m1[:, f, :], pss[j][:], AF.Gelu_apprx_tanh,
                    bias=vecs_sb[:, VC_BM1 + f : VC_BM1 + f + 1],
                    scale=vecs_sb[:, VC_S1 + f : VC_S1 + f + 1],
                )
        cm_ps5.__exit__(None, None, None)
        cm_w5.__exit__(None, None, None)
        cm_h8.__exit__(None, None, None)

        # ---------------- phase 6: MLP layer 2 + residual --------------
        cm_t2 = tc.tile_pool(name="p_t2", bufs=1)
        t2 = cm_t2.__enter__().tile([P, DC, NXP], F32R, name="t2")
        cm_w6 = tc.tile_pool(name="w6", bufs=6)
        wpool6 = cm_w6.__enter__()
        cm_st6 = tc.tile_pool(name="st6", bufs=3)
        stage6 = cm_st6.__enter__()
        cm_ps6 = tc.tile_pool(name="ps6", bufs=8, space="PSUM")
        psp6 = cm_ps6.__enter__()
        KD = DH // (2 * P)  # 32 double-row contraction steps
        for fg in range(2):
            pss = [
                psp6.tile([P, 512], F32, tag="m2p", name=f"m2p{fg}_{j}")
                for j in range(8)
            ]
            for k2 in range(KD):
                wt = wpool6.tile([P, 2, 1024], FP8, tag="wm2", name="wm2t")
                nc.sync.dma_start(
                    wt[:],
                    wm2[
                        k2 * 2 * P : (k2 + 1) * 2 * P,
                        fg * 1024 : (fg + 1) * 1024,
                    ].rearrange("(h p) c -> p h c", p=P),
                )
                for j in range(8):
                    nc.tensor.matmul(
                        pss[j][:],
                        lhsT=wt[:, :, j * P : (j + 1) * P],
                        rhs=m1[:, 2 * k2 : 2 * k2 + 2, :],
                        start=(k2 == 0),
                        stop=(k2 == KD - 1),
                        perf_mode=mybir.MatmulPerfMode.DoubleRow,
                    )
            for j in range(8):
                f = fg * 8 + j
                tsc = stage6.tile([P, NXP], F32, tag="m2s", name="m2s")
                nc.vector.tensor_scalar(
                    tsc[:], pss[j][:],
                    vecs_sb[:, VC_S2 + f : VC_S2 + f + 1],
                    vecs_sb[:, VC_BM2 + f : VC_BM2 + f + 1],
                    OP.mult, OP.add,
                )
                nc.vector.tensor_add(t2[:, f, :], tsc[:], h_sb[:, f, :])
        if DEBUG:
            nc.sync.dma_start(dbg["dt2"][:], t2[:])
        cm_ps6.__exit__(None, None, None)
        cm_st6.__exit__(None, None, None)
        cm_w6.__exit__(None, None, None)
        cm_m1.__exit__(None, None, None)
        cm_h.__exit__(None, None, None)

        # ---------------- phase 7: LN2 + output ------------------------
        cm_st7 = tc.tile_pool(name="st7", bufs=4)
        stage7 = cm_st7.__enter__()

        def emit_out(f, tmp):
            ost = stage7.tile([P, NXP], F32, tag="ost", name="ost")
            nc.scalar.activation(
                ost[:], tmp[:], AF.Identity,
                bias=vecs_sb[:, VC_B2 + f : VC_B2 + f + 1],
                scale=vecs_sb[:, VC_G2 + f : VC_G2 + f + 1],
            )
            nc.sync.dma_start(out[:, f, 0:NXP], ost[:])

        layernorm(t2, emit_out)
        cm_st7.__exit__(None, None, None)
        cm_t2.__exit__(None, None, None)
        cm_dram.__exit__(None, None, None)
        cm_const.__exit__(None, None, None)

    nc.compile()
    return nc


def _get_nc():
    global _NC
    if _NC is None:
        _NC = _build()
    return _NC


def _chunkT(a, dtype):
    """[n, 2048] token-major -> [128, 16, n] feature-chunked transpose."""
    n = a.shape[0]
    return np.ascontiguousarray(
        a.T.reshape(DC, P, n).transpose(1, 0, 2).astype(dtype)
    )


def _packvec(v):
    return np.asarray(v, np.float32).reshape(-1, P).T  # [128, k]


def kernel(**inputs):
    inputs = {k: np.asarray(v) for k, v in inputs.items()}
    x = inputs["x"].astype(np.float32)
    state = inputs["state"].astype(np.float32)
    Wq, bq = inputs["Wq"], inputs["bq"]
    Wk, bk = inputs["Wk"], inputs["bk"]
    Wv, bv = inputs["Wv"], inputs["bv"]
    Wo1, bo1 = inputs["Wo1"], inputs["bo1"]
    Wo2, bo2 = inputs["Wo2"], inputs["bo2"]
    Wm1, bm1 = inputs["Wm1"], inputs["bm1"]
    Wm2, bm2 = inputs["Wm2"], inputs["bm2"]

    cbias = (bv @ Wo1 + bo1) @ Wo2 + bo2
    s2 = np.abs(Wm2).max(axis=0).astype(np.float32) / 224.0 + 1e-30
    wm2_fp8 = np.ascontiguousarray(
        (Wm2 / s2[None, :]).astype(ml_dtypes.float8_e4m3)
    )
    s1 = np.abs(Wm1).max(axis=0).astype(np.float32) / 224.0 + 1e-30
    wm1_fp8 = np.ascontiguousarray(
        (Wm1 / s1[None, :]).astype(ml_dtypes.float8_e4m3)
    )
    vecs = np.concatenate(
        [
            _packvec(bq), _packvec(bk), _packvec(bm1), _packvec(bm2),
            _packvec(cbias),
            _packvec(inputs["ln1_g"]), _packvec(inputs["ln1_b"]),
            _packvec(inputs["ln2_g"]), _packvec(inputs["ln2_b"]),
            _packvec(s2),
            _packvec(s1),
        ],
        axis=1,
    )
    vecs = np.ascontiguousarray(vecs, dtype=np.float32)

    bfc = lambda w: np.ascontiguousarray(np.asarray(w).astype(ml_dtypes.bfloat16))
    common = {
        "wq": bfc(Wq), "wk": bfc(Wk), "wv": bfc(Wv),
        "wo1": bfc(Wo1), "wo2": bfc(Wo2),
        "wm1": wm1_fp8, "wm2": wm2_fp8,
        "vecs": vecs,
    }

    kpos = np.arange(T)
    in_maps = []
    for c in range(8):
        b, half = divmod(c, 2)
        if half == 0:
            qtok = np.concatenate([x[b, 0:512], state[b, 0:128]], axis=0)
            qpos = np.concatenate([np.arange(256, 768), np.arange(1280, 1408)])
            kvtok = np.concatenate([state[b], x[b, 0:384]], axis=0)
        else:
            qtok = np.concatenate([x[b, 512:1024], state[b, 128:256]], axis=0)
            qpos = np.concatenate([np.arange(768, 1280), np.arange(1408, 1536)])
            kvtok = x[b, 384:1024]
        maskT = (kpos[:, None] <= qpos[None, :]).astype(ml_dtypes.bfloat16)
        maskT = np.ascontiguousarray(maskT.reshape(KCN, P, NQ).transpose(1, 0, 2))
        in_maps.append(
            dict(
                common,
                xq=_chunkT(qtok, ml_dtypes.bfloat16),
                xres=_chunkT(qtok[:NXP], np.float32),
                kvt=_chunkT(kvtok, ml_dtypes.bfloat16),
                maskt=maskT,
            )
        )

    nc = _get_nc()
    res = run_bass_kernel_spmd(nc, in_maps, core_ids=list(range(8)))
    global LAST_RESULT
    LAST_RESULT = res

    out = np.empty((B, L, D), np.float32)
    state_next = np.empty((B, S, D), np.float32)
    for c in range(8):
        b, half = divmod(c, 2)
        oc = np.asarray(res.results[c]["out"])  # [128, 16, 640]
        ocT = oc.transpose(1, 0, 2).reshape(D, NQ).T  # [640, 2048]
        out[b, half * 512 : (half + 1) * 512] = ocT[0:512]
        state_next[b, half * 128 : (half + 1) * 128] = ocT[512:640]
    return out, state_next


if __name__ == "__main__":
    _build()
    print("build ok")


# revision 20
# speedup vs baseline: 1.1088x; 1.1088x over previous
"""ARCformer block on 8 TRN2 NeuronCores.

Sharding: data-parallel over batch (4) x 2-way split of the sequence
positions within each batch pair. Each core handles 640 of the 1280
"needed" query positions of one batch (512 x-positions + 128 state
positions) and computes K/V for 640 of the 1280 unique tokens; K/V
shards are exchanged with the pair partner via 4 chunked pair
AllGathers (one per 4-head group) that overlap the Q projection and
attention.

On-core layout: activations are kept feature-major (feature on the
SBUF partition axis, tokens on the free axis), so every linear layer
is lhsT=W-chunk @ rhs=X^T-chunk with no transposes. Attention runs in
the "scores transposed" orientation [keys, queries]: softmax
numerators via ACT exp (max-free; |scores| <= ~8 for this problem),
causal mask applied as a 0/1 multiply, denominators via an all-ones
matmul accumulated in PSUM alongside P@V, normalization folded into
the PSUM drain. LayerNorm stats are computed with ones-matmuls
(pre-broadcast across partitions); rstd = exp(-0.5*ln(var+eps)) keeps
ACT in the natural_log_exp table set. All GEMMs run in bf16 (weights
pre-cast on the host) with fp32 PSUM accumulation; LayerNorm sums and
the residual chain stay in fp32/f32r.
"""

import numpy as np
import ml_dtypes

import concourse.bass as bass
import concourse.tile as tile
from concourse import bacc, mybir
from concourse.bass_utils import run_bass_kernel_spmd
from concourse.masks import make_identity

F32 = mybir.dt.float32
F32R = mybir.dt.float32r
FP8 = mybir.dt.float8e4
BF16 = mybir.dt.bfloat16
AF = mybir.ActivationFunctionType
OP = mybir.AluOpType

B, L, S, D = 4, 1024, 256, 2048
H, DK, DV = 16, 128, 128
DH = 8192
T = S + L + S  # 1536
P = 128
DC = D // P  # 16
NQ = 640  # per-core query positions (512 x-part + 128 state-part)
NXP = 512  # x-part columns
NKV = 640  # per-core unique kv tokens
KCN = 12  # key chunks (1536/128)
EPS = 1e-5
ISQ = float(1.0 / np.sqrt(DK))

# attention spans: (q0, qn, klim, mlim)
SPANS = [(0, 512, 10, 2), (512, 128, 12, 10)]
# projection token spans
PSPANS = [(0, 512), (512, 128)]

# vecs columns
VC_BQ, VC_BK, VC_BM1, VC_BM2, VC_CB = 0, 16, 32, 96, 112
VC_G1, VC_B1, VC_G2, VC_B2 = 128, 144, 160, 176
VC_S2 = 192
VC_S1 = 208
VC_N = 272

_NC = None
LAST_RESULT = None
DEBUG = False


def _build():
    nc = bacc.Bacc("TRN2", target_bir_lowering=False, debug=False, num_devices=8)

    def inp(name, shape, dt):
        return nc.dram_tensor(name, shape, dt, kind="ExternalInput").ap()

    xq = inp("xq", [P, DC, NQ], BF16)
    xres = inp("xres", [P, DC, NXP], F32)
    kvt = inp("kvt", [P, DC, NKV], BF16)
    maskt = inp("maskt", [P, KCN, NQ], BF16)
    wq = inp("wq", [D, D], BF16)
    wk = inp("wk", [D, D], BF16)
    wv = inp("wv", [D, D], BF16)
    wo1 = inp("wo1", [D, D], BF16)
    wo2 = inp("wo2", [D, D], BF16)
    wm1 = inp("wm1", [D, DH], FP8)
    wm2 = inp("wm2", [DH, D], FP8)
    vecs = inp("vecs", [P, VC_N], F32)
    out = nc.dram_tensor("out", [P, DC, NQ], F32, kind="ExternalOutput").ap()
    dbg = {}
    if DEBUG:
        dbg["dq"] = nc.dram_tensor("dq", [P, DC, NQ], BF16, kind="ExternalOutput").ap()
        dbg["dkt"] = nc.dram_tensor("dkt", [P, T], BF16, kind="ExternalOutput").ap()
        dbg["dvt"] = nc.dram_tensor("dvt", [P, KCN, P], BF16, kind="ExternalOutput").ap()
        dbg["det"] = nc.dram_tensor("det", [P, KCN, NQ], BF16, kind="ExternalOutput").ap()
        dbg["drt"] = nc.dram_tensor("drt", [P, 2, 512], F32, kind="ExternalOutput").ap()
        dbg["do"] = nc.dram_tensor("do", [P, DC, NQ], BF16, kind="ExternalOutput").ap()
        dbg["dt"] = nc.dram_tensor("dt", [P, DC, NXP], F32R, kind="ExternalOutput").ap()
        dbg["dh"] = nc.dram_tensor("dh", [P, DC, NXP], BF16, kind="ExternalOutput").ap()
        dbg["dt2"] = nc.dram_tensor("dt2", [P, DC, NXP], F32R, kind="ExternalOutput").ap()

    with tile.TileContext(nc, pool_alloc_mode="queue") as tc:
        cm_const = tc.tile_pool(name="const", bufs=1)
        const = cm_const.__enter__()
        vecs_sb = const.tile([P, VC_N], F32)
        nc.sync.dma_start(vecs_sb[:], vecs)
        ones_f = const.tile([P, P], F32)
        nc.vector.memset(ones_f[:], 1.0)
        ones_r = const.tile([P, P], F32R)
        nc.vector.tensor_copy(ones_r[:], ones_f[:])
        ones_bf = const.tile([P, P], BF16)
        nc.vector.memset(ones_bf[:], 1.0)
        eps_sb = const.tile([P, 1], F32)
        nc.vector.memset(eps_sb[:], EPS)
        ident_bf = const.tile([P, P], BF16)
        make_identity(nc, ident_bf[:])

        cm_dram = tc.tile_pool(name="dram", bufs=1, space="DRAM")
        dram = cm_dram.__enter__()
        kvb_in = [
            dram.tile([2, 4 * P, NKV], BF16, name=f"kvbin{g}") for g in range(4)
        ]
        kvb_out = [
            dram.tile([2, 2, 4 * P, NKV], BF16, name=f"kvbout{g}") for g in range(4)
        ]

        cm_qm = tc.tile_pool(name="p_qm", bufs=1)
        p_qm = cm_qm.__enter__()
        q_sb = p_qm.tile([P, DC, NQ], BF16, name="q_sb")
        mask_sb = p_qm.tile([P, KCN, NQ], BF16, name="mask_sb")

        # ---------------- phase 1: QKV projections + pair AllGather ----
        # 4 f-chunks per weight tile [128, 512]; PSUM [128, 2, 512] per f.
        def proj(w_ap, src_sb, fgs, drain, psp, wpool, ncols):
            for fg in fgs:
                pss = [
                    psp.tile([P, 2, 512], F32, tag="pp", name=f"pp{fg}_{j}")
                    for j in range(4)
                ]
                for d in range(DC):
                    wt = wpool.tile([P, 512], BF16, tag="w", name="wt")
                    nc.sync.dma_start(
                        wt[:], w_ap[d * P : (d + 1) * P, fg * 512 : (fg + 1) * 512]
                    )
                    for j in range(4):
                        for s, (s0, sn) in enumerate(PSPANS):
                            if s0 >= ncols:
                                continue
                            nc.tensor.matmul(
                                pss[j][:, s, :sn],
                                lhsT=wt[:, j * P : (j + 1) * P],
                                rhs=src_sb[:, d, s0 : s0 + sn],
                                start=(d == 0),
                                stop=(d == DC - 1),
                            )
                for j in range(4):
                    drain(fg * 4 + j, pss[j])

        cm_in = tc.tile_pool(name="p_in", bufs=1)
        p_in = cm_in.__enter__()
        xq_sb = p_in.tile([P, DC, NQ], BF16, name="xq_sb")
        kv_sb = p_in.tile([P, DC, NKV], BF16, name="kv_sb")
        for d in range(DC):
            nc.sync.dma_start(kv_sb[:, d, :], kvt[:, d, :])
        for d in range(DC):
            nc.sync.dma_start(xq_sb[:, d, :], xq[:, d, :])
        nc.sync.dma_start(mask_sb[:], maskt)

        cm_w1 = tc.tile_pool(name="w1", bufs=10)
        wpool1 = cm_w1.__enter__()
        cm_st1 = tc.tile_pool(name="st1", bufs=4)
        stage1 = cm_st1.__enter__()
        cm_ps1 = tc.tile_pool(name="ps1", bufs=4, space="PSUM")
        psp1 = cm_ps1.__enter__()

        def drain_kv(which, f, ps):
            st = stage1.tile([P, NKV], BF16, tag="kvst", name="kvst")
            for s, (s0, sn) in enumerate(PSPANS):
                if which == 0:
                    nc.scalar.activation(
                        st[:, s0 : s0 + sn],
                        ps[:, s, :sn],
                        AF.Identity,
                        bias=vecs_sb[:, VC_BK + f : VC_BK + f + 1],
                    )
                else:
                    nc.scalar.activation(st[:, s0 : s0 + sn], ps[:, s, :sn], AF.Copy)
            g, r = f // 4, f % 4
            nc.sync.dma_start(kvb_in[g][which, r * P : (r + 1) * P, :], st[:])

        def drain_q(f, ps):
            for s, (s0, sn) in enumerate(PSPANS):
                nc.scalar.activation(
                    q_sb[:, f, s0 : s0 + sn],
                    ps[:, s, :sn],
                    AF.Identity,
                    bias=vecs_sb[:, VC_BQ + f : VC_BQ + f + 1],
                )

        for g in range(4):
            proj(wk, kv_sb, [g], lambda f, ps: drain_kv(0, f, ps), psp1, wpool1, NKV)
            proj(wv, kv_sb, [g], lambda f, ps: drain_kv(1, f, ps), psp1, wpool1, NKV)
            nc.gpsimd.collective_compute(
                "AllGather",
                OP.bypass,
                replica_groups=[[0, 1], [2, 3], [4, 5], [6, 7]],
                ins=[kvb_in[g].opt()],
                outs=[kvb_out[g].opt()],
            )
        proj(wq, xq_sb, list(range(4)), drain_q, psp1, wpool1, NQ)
        if DEBUG:
            nc.sync.dma_start(dbg["dq"][:], q_sb[:])

        cm_ps1.__exit__(None, None, None)
        cm_st1.__exit__(None, None, None)
        cm_w1.__exit__(None, None, None)
        cm_in.__exit__(None, None, None)

        # ---------------- phase 2: attention ---------------------------
        cm_o = tc.tile_pool(name="p_o", bufs=1, side="right")
        o_sb = cm_o.__enter__().tile([P, DC, NQ], BF16, name="o_sb")

        # pre-pass: transpose all V chunks to token-major [nk, DV]
        cm_va = tc.tile_pool(name="p_va", bufs=1)
        vt_all = cm_va.__enter__().tile([P, H, KCN, P], BF16, name="vt_all")
        cm_vp = tc.tile_pool(name="vp", bufs=4)
        vpool = cm_vp.__enter__()
        cm_apt = tc.tile_pool(name="apt", bufs=4, space="PSUM")
        aps_t = cm_apt.__enter__()
        for hh in range(H):
            g, hr = hh // 4, hh % 4
            vts = vpool.tile([P, T], BF16, tag="vts", name="vts")
            nc.sync.dma_start(
                vts[:, 0:640], kvb_out[g][0, 1, hr * P : (hr + 1) * P, :]
            )
            nc.sync.dma_start(
                vts[:, 640:1280], kvb_out[g][1, 1, hr * P : (hr + 1) * P, :]
            )
            nc.sync.dma_start(
                vts[:, 1280:1536], kvb_out[g][0, 1, hr * P : (hr + 1) * P, 0:256]
            )
            for kc in range(KCN):
                tp = aps_t.tile([P, P], BF16, tag="tp", name="tp")
                nc.tensor.transpose(
                    tp[:], vts[:, kc * P : (kc + 1) * P], ident_bf[:]
                )
                nc.vector.tensor_copy(vt_all[:, hh, kc, :], tp[:])
        cm_apt.__exit__(None, None, None)
        cm_vp.__exit__(None, None, None)

        cm_kp = tc.tile_pool(name="kp", bufs=2)
        kpool = cm_kp.__enter__()
        cm_ep = tc.tile_pool(name="ep", bufs=2)
        epool = cm_ep.__enter__()
        cm_rp = tc.tile_pool(name="rp", bufs=2)
        rpool = cm_rp.__enter__()
        cm_aps = tc.tile_pool(name="aps", bufs=2, space="PSUM")
        aps_s = cm_aps.__enter__()
        cm_apo = tc.tile_pool(name="apo", bufs=1, space="PSUM")
        aps_o = cm_apo.__enter__()
        cm_apu = tc.tile_pool(name="apu", bufs=1, space="PSUM")
        aps_u = cm_apu.__enter__()

        def attn_front(hh):
            g, hr = hh // 4, hh % 4
            kt = kpool.tile([P, T], BF16, tag="kt", name="kt")
            nc.sync.dma_start(kt[:, 0:640], kvb_out[g][0, 0, hr * P : (hr + 1) * P, :])
            nc.sync.dma_start(
                kt[:, 640:1280], kvb_out[g][1, 0, hr * P : (hr + 1) * P, :]
            )
            nc.sync.dma_start(
                kt[:, 1280:1536], kvb_out[g][0, 0, hr * P : (hr + 1) * P, 0:256]
            )
            et = epool.tile([P, KCN, NQ], BF16, tag="et", name="et")
            for s, (q0, qn, klim, mlim) in enumerate(SPANS):
                for kc in range(0, klim, 2):
                    pss = aps_s.tile([P, 2, 512], F32, tag="ps_s", name="ps_s")
                    for u in range(2):
                        nc.tensor.matmul(
                            pss[:, u, :qn],
                            lhsT=kt[:, (kc + u) * P : (kc + u + 1) * P],
                            rhs=q_sb[:, hh, q0 : q0 + qn],
                            start=True,
                            stop=True,
                        )
                    nc.scalar.activation(
                        et[:, kc : kc + 2, q0 : q0 + qn],
                        pss[:, :, :qn],
                        AF.Exp,
                        scale=ISQ,
                    )
                    if kc >= mlim:
                        nc.vector.tensor_mul(
                            et[:, kc : kc + 2, q0 : q0 + qn],
                            et[:, kc : kc + 2, q0 : q0 + qn],
                            mask_sb[:, kc : kc + 2, q0 : q0 + qn],
                        )
            return et

        def attn_back(hh, et):
            vt = vt_all[:, hh]
            ps_o = aps_o.tile([P, 2, 512], F32, tag="ps_o", name="ps_o")
            ps_u = aps_u.tile([P, 2, 512], F32, tag="ps_u", name="ps_u")
            for s, (q0, qn, klim, _) in enumerate(SPANS):
                for kc in range(klim):
                    nc.tensor.matmul(
                        ps_u[:, s, :qn],
                        lhsT=ones_bf[:],
                        rhs=et[:, kc, q0 : q0 + qn],
                        start=(kc == 0),
                        stop=(kc == klim - 1),
                    )
                for kc in range(klim):
                    nc.tensor.matmul(
                        ps_o[:, s, :qn],
                        lhsT=vt[:, kc, :],
                        rhs=et[:, kc, q0 : q0 + qn],
                        start=(kc == 0),
                        stop=(kc == klim - 1),
                    )
            rt = rpool.tile([P, 2, 512], F32, tag="rt", name="rt")
            for s, (q0, qn, _, _) in enumerate(SPANS):
                nc.vector.reciprocal_approx_fast(rt[:, s, :qn], ps_u[:, s, :qn])
                nc.vector.tensor_mul(
                    o_sb[:, hh, q0 : q0 + qn], ps_o[:, s, :qn], rt[:, s, :qn]
                )

        for hh in range(H):
            et = attn_front(hh)
            attn_back(hh, et)
        if DEBUG:
            nc.sync.dma_start(dbg["do"][:], o_sb[:])

        cm_apu.__exit__(None, None, None)
        cm_apo.__exit__(None, None, None)
        cm_aps.__exit__(None, None, None)
        cm_rp.__exit__(None, None, None)
        cm_ep.__exit__(None, None, None)
        cm_kp.__exit__(None, None, None)
        cm_va.__exit__(None, None, None)
        cm_qm.__exit__(None, None, None)

        # ---------------- phase 3: output projection -------------------
        cm_t = tc.tile_pool(name="p_t", bufs=1)
        t_sb = cm_t.__enter__().tile([P, DC, NXP], F32R, name="t_sb")
        cm_u1 = tc.tile_pool(name="p_u1", bufs=1)
        u1 = cm_u1.__enter__().tile([P, DC, NQ], BF16, name="u1")
        cm_w3 = tc.tile_pool(name="w3", bufs=10)
        wpool3 = cm_w3.__enter__()
        cm_st3 = tc.tile_pool(name="st3", bufs=4)
        stage3 = cm_st3.__enter__()
        cm_ps3 = tc.tile_pool(name="ps3", bufs=4, space="PSUM")
        psp3 = cm_ps3.__enter__()

        def drain_u1(f, ps):
            for s, (s0, sn) in enumerate(PSPANS):
                nc.scalar.activation(u1[:, f, s0 : s0 + sn], ps[:, s, :sn], AF.Copy)

        proj(wo1, o_sb, list(range(4)), drain_u1, psp3, wpool3, NQ)

        def drain_u2(f, ps):
            cb = vecs_sb[:, VC_CB + f : VC_CB + f + 1]
            xst = stage3.tile([P, NXP], F32, tag="xst", name="xst")
            nc.sync.dma_start(xst[:], xres[:, f, :])
            nc.vector.scalar_tensor_tensor(
                t_sb[:, f, :], ps[:, 0, :512], cb, xst[:], OP.add, OP.add
            )
            sst = stage3.tile([P, P], F32, tag="sst", name="sst")
            nc.vector.tensor_scalar_add(sst[:], ps[:, 1, :128], cb)
            nc.sync.dma_start(out[:, f, NXP:NQ], sst[:])

        proj(wo2, u1, list(range(4)), drain_u2, psp3, wpool3, NQ)
        if DEBUG:
            nc.sync.dma_start(dbg["dt"][:], t_sb[:])

        cm_ps3.__exit__(None, None, None)
        cm_st3.__exit__(None, None, None)
        cm_w3.__exit__(None, None, None)
        cm_u1.__exit__(None, None, None)
        cm_o.__exit__(None, None, None)

        # ---------------- layernorm helper -----------------------------
        def layernorm(src, emit):
            cm_sq = tc.tile_pool(name="lnsq", bufs=1)
            sq = cm_sq.__enter__().tile([P, DC, NXP], F32R, name="sq")
            cm_stat = tc.tile_pool(name="lnstat", bufs=1)
            statp = cm_stat.__enter__()
            cm_tmp = tc.tile_pool(name="lntmp", bufs=3)
            tmpp = cm_tmp.__enter__()
            cm_psl = tc.tile_pool(name="lnps", bufs=2, space="PSUM")
            pslp = cm_psl.__enter__()

            for f in range(DC):
                nc.vector.tensor_mul(sq[:, f, :], src[:, f, :], src[:, f, :])
            ps_m = pslp.tile([P, 512], F32, tag="lm", name="lnm")
            ps_v = pslp.tile([P, 512], F32, tag="lv", name="lnv")
            for d in range(DC):
                nc.tensor.matmul(
                    ps_m[:, :NXP], lhsT=ones_r[:], rhs=src[:, d, :],
                    start=(d == 0), stop=(d == DC - 1),
                )
            for d in range(DC):
                nc.tensor.matmul(
                    ps_v[:, :NXP], lhsT=ones_r[:], rhs=sq[:, d, :],
                    start=(d == 0), stop=(d == DC - 1),
                )
            mean = statp.tile([P, NXP], F32, name="mean")
            nc.vector.tensor_scalar_mul(mean[:], ps_m[:, :NXP], 1.0 / D)
            m2 = statp.tile([P, NXP], F32, name="m2")
            nc.vector.tensor_mul(m2[:], mean[:], mean[:])
            var = statp.tile([P, NXP], F32, name="var")
            nc.vector.scalar_tensor_tensor(
                var[:], ps_v[:, :NXP], 1.0 / D, m2[:], OP.mult, OP.subtract
            )
            lnv = statp.tile([P, NXP], F32, name="lnv2")
            nc.scalar.activation(lnv[:], var[:], AF.Ln, bias=eps_sb[:])
            rstd = statp.tile([P, NXP], F32, name="rstd")
            nc.scalar.activation(rstd[:], lnv[:], AF.Exp, scale=-0.5)
            for f in range(DC):
                tmp = tmpp.tile([P, NXP], F32, tag="lnt", name="lnt")
                nc.vector.tensor_sub(tmp[:], src[:, f, :], mean[:])
                nc.vector.tensor_mul(tmp[:], tmp[:], rstd[:])
                emit(f, tmp)

            cm_psl.__exit__(None, None, None)
            cm_tmp.__exit__(None, None, None)
            cm_stat.__exit__(None, None, None)
            cm_sq.__exit__(None, None, None)

        # ---------------- phase 4: LN1 ---------------------------------
        cm_h = tc.tile_pool(name="p_h", bufs=1, side="right")
        h_sb = cm_h.__enter__().tile([P, DC, NXP], BF16, name="h_sb")

        def emit_h(f, tmp):
            nc.scalar.activation(
                h_sb[:, f, :], tmp[:], AF.Identity,
                bias=vecs_sb[:, VC_B1 + f : VC_B1 + f + 1],
                scale=vecs_sb[:, VC_G1 + f : VC_G1 + f + 1],
            )

        layernorm(t_sb, emit_h)
        if DEBUG:
            nc.sync.dma_start(dbg["dh"][:], h_sb[:])
        cm_t.__exit__(None, None, None)
        cm_h8 = tc.tile_pool(name="p_h8", bufs=1)
        h8 = cm_h8.__enter__().tile([P, DC, NXP], FP8, name="h8")
        for f in range(DC):
            nc.vector.tensor_copy(h8[:, f, :], h_sb[:, f, :])

        # ---------------- phase 5: MLP layer 1 -------------------------
        cm_m1 = tc.tile_pool(name="p_m1", bufs=1, side="right")
        m1 = cm_m1.__enter__().tile([P, DH // P, NXP], FP8, name="m1")
        cm_w5 = tc.tile_pool(name="w5", bufs=6)
        wpool5 = cm_w5.__enter__()
        cm_ps5 = tc.tile_pool(name="ps5", bufs=8, space="PSUM")
        psp5 = cm_ps5.__enter__()
        KD1 = DC // 2  # 8 double-row contraction steps
        for fg in range(8):
            pss = [
                psp5.tile([P, 512], F32, tag="mp", name=f"mp{fg}_{j}")
                for j in range(8)
            ]
            for k2 in range(KD1):
                wt = wpool5.tile([P, 2, 1024], FP8, tag="wm1", name="wm1t")
                nc.sync.dma_start(
                    wt[:],
                    wm1[
                        k2 * 2 * P : (k2 + 1) * 2 * P,
                        fg * 1024 : (fg + 1) * 1024,
                    ].rearrange("(h p) c -> p h c", p=P),
                )
                for j in range(8):
                    nc.tensor.matmul(
                        pss[j][:],
                        lhsT=wt[:, :, j * P : (j + 1) * P],
                        rhs=h8[:, 2 * k2 : 2 * k2 + 2, :],
                        start=(k2 == 0),
                        stop=(k2 == KD1 - 1),
                        perf_mode=mybir.MatmulPerfMode.DoubleRow,
                    )
            for j in range(8):
                f = fg * 8 + j
                nc.scalar.activation(
                    m1[:, f, :], pss[j][:], AF.Gelu_apprx_tanh,
                    bias=vecs_sb[:, VC_BM1 + f : VC_BM1 + f + 1],
                    scale=vecs_sb[:, VC_S1 + f : VC_S1 + f + 1],
                )
        cm_ps5.__exit__(None, None, None)
        cm_w5.__exit__(None, None, None)
        cm_h8.__exit__(None, None, None)

        # ---------------- phase 6: MLP layer 2 + residual --------------
        cm_t2 = tc.tile_pool(name="p_t2", bufs=1)
        t2 = cm_t2.__enter__().tile([P, DC, NXP], F32R, name="t2")
        cm_w6 = tc.tile_pool(name="w6", bufs=6)
        wpool6 = cm_w6.__enter__()
        cm_st6 = tc.tile_pool(name="st6", bufs=3)
        stage6 = cm_st6.__enter__()
        cm_ps6 = tc.tile_pool(name="ps6", bufs=8, space="PSUM")
        psp6 = cm_ps6.__enter__()
        KD = DH // (2 * P)  # 32 double-row contraction steps
        for fg in range(2):
            pss = [
                psp6.tile([P, 512], F32, tag="m2p", name=f"m2p{fg}_{j}")
                for j in range(8)
            ]
            for k2 in range(KD):
                wt = wpool6.tile([P, 2, 1024], FP8, tag="wm2", name="wm2t")
                nc.sync.dma_start(
                    wt[:],
                    wm2[
                        k2 * 2 * P : (k2 + 1) * 2 * P,
                        fg * 1024 : (fg + 1) * 1024,
                    ].rearrange("(h p) c -> p h c", p=P),
                )
                for j in range(8):
                    nc.tensor.matmul(
                        pss[j][:],
                        lhsT=wt[:, :, j * P : (j + 1) * P],
                        rhs=m1[:, 2 * k2 : 2 * k2 + 2, :],
                        start=(k2 == 0),
                        stop=(k2 == KD - 1),
                        perf_mode=mybir.MatmulPerfMode.DoubleRow,
                    )
            for j in range(8):
                f = fg * 8 + j
                tsc = stage6.tile([P, NXP], F32, tag="m2s", name="m2s")
                nc.vector.tensor_scalar(
                    tsc[:], pss[j][:],
                    vecs_sb[:, VC_S2 + f : VC_S2 + f + 1],
                    vecs_sb[:, VC_BM2 + f : VC_BM2 + f + 1],
                    OP.mult, OP.add,
                )
                nc.vector.tensor_add(t2[:, f, :], tsc[:], h_sb[:, f, :])
        if DEBUG:
            nc.sync.dma_start(dbg["dt2"][:], t2[:])
        cm_ps6.__exit__(None, None, None)
        cm_st6.__exit__(None, None, None)
        cm_w6.__exit__(None, None, None)
        cm_m1.__exit__(None, None, None)
        cm_h.__exit__(None, None, None)

        # ---------------- phase 7: LN2 + output ------------------------
        cm_st7 = tc.tile_pool(name="st7", bufs=4)
        stage7 = cm_st7.__enter__()

        def emit_out(f, tmp):
            ost = stage7.tile([P, NXP], F32, tag="ost", name="ost")
            nc.scalar.activation(
                ost[:], tmp[:], AF.Identity,
                bias=vecs_sb[:, VC_B2 + f : VC_B2 + f + 1],
                scale=vecs_sb[:, VC_G2 + f : VC_G2 + f + 1],
            )
            nc.sync.dma_start(out[:, f, 0:NXP], ost[:])

        layernorm(t2, emit_out)
        cm_st7.__exit__(None, None, None)
        cm_t2.__exit__(None, None, None)
        cm_dram.__exit__(None, None, None)
        cm_const.__exit__(None, None, None)

    nc.compile()
    return nc


def _get_nc():
    global _NC
    if _NC is None:
        _NC = _build()
    return _NC


def _chunkT(a, dtype):
    """[n, 2048] token-major -> [128, 16, n] feature-chunked transpose."""
    n = a.shape[0]
    return np.ascontiguousarray(
        a.T.reshape(DC, P, n).transpose(1, 0, 2).astype(dtype)
    )


def _packvec(v):
    return np.asarray(v, np.float32).reshape(-1, P).T  # [128, k]


def kernel(**inputs):
    inputs = {k: np.asarray(v) for k, v in inputs.items()}
    x = inputs["x"].astype(np.float32)
    state = inputs["state"].astype(np.float32)
    Wq, bq = inputs["Wq"], inputs["bq"]
    Wk, bk = inputs["Wk"], inputs["bk"]
    Wv, bv = inputs["Wv"], inputs["bv"]
    Wo1, bo1 = inputs["Wo1"], inputs["bo1"]
    Wo2, bo2 = inputs["Wo2"], inputs["bo2"]
    Wm1, bm1 = inputs["Wm1"], inputs["bm1"]
    Wm2, bm2 = inputs["Wm2"], inputs["bm2"]

    cbias = (bv @ Wo1 + bo1) @ Wo2 + bo2
    s2 = np.abs(Wm2).max(axis=0).astype(np.float32) / 224.0 + 1e-30
    wm2_fp8 = np.ascontiguousarray(
        (Wm2 / s2[None, :]).astype(ml_dtypes.float8_e4m3)
    )
    s1 = np.abs(Wm1).max(axis=0).astype(np.float32) / 224.0 + 1e-30
    wm1_fp8 = np.ascontiguousarray(
        (Wm1 / s1[None, :]).astype(ml_dtypes.float8_e4m3)
    )
    vecs = np.concatenate(
        [
            _packvec(bq), _packvec(bk), _packvec(bm1), _packvec(bm2),
            _packvec(cbias),
            _packvec(inputs["ln1_g"]), _packvec(inputs["ln1_b"]),
            _packvec(inputs["ln2_g"]), _packvec(inputs["ln2_b"]),
            _packvec(s2),
            _packvec(s1),
        ],
        axis=1,
    )
    vecs = np.ascontiguousarray(vecs, dtype=np.float32)

    bfc = lambda w: np.ascontiguousarray(np.asarray(w).astype(ml_dtypes.bfloat16))
    common = {
        "wq": bfc(Wq), "wk": bfc(Wk), "wv": bfc(Wv),
        "wo1": bfc(Wo1), "wo2": bfc(Wo2),
        "wm1": wm1_fp8, "wm2": wm2_fp8,
        "vecs": vecs,
    }

    kpos = np.arange(T)
    in_maps = []
    for c in range(8):
        b, half = divmod(c, 2)
        if half == 0:
            qtok = np.concatenate([x[b, 0:512], state[b, 0:128]], axis=0)
            qpos = np.concatenate([np.arange(256, 768), np.arange(1280, 1408)])
            kvtok = np.concatenate([state[b], x[b, 0:384]], axis=0)
        else:
            qtok = np.concatenate([x[b, 512:1024], state[b, 128:256]], axis=0)
            qpos = np.concatenate([np.arange(768, 1280), np.arange(1408, 1536)])
            kvtok = x[b, 384:1024]
        maskT = (kpos[:, None] <= qpos[None, :]).astype(ml_dtypes.bfloat16)
        maskT = np.ascontiguousarray(maskT.reshape(KCN, P, NQ).transpose(1, 0, 2))
        in_maps.append(
            dict(
                common,
                xq=_chunkT(qtok, ml_dtypes.bfloat16),
                xres=_chunkT(qtok[:NXP], np.float32),
                kvt=_chunkT(kvtok, ml_dtypes.bfloat16),
                maskt=maskT,
            )
        )

    nc = _get_nc()
    res = run_bass_kernel_spmd(nc, in_maps, core_ids=list(range(8)))
    global LAST_RESULT
    LAST_RESULT = res

    out = np.empty((B, L, D), np.float32)
    state_next = np.empty((B, S, D), np.float32)
    for c in range(8):
        b, half = divmod(c, 2)
        oc = np.asarray(res.results[c]["out"])  # [128, 16, 640]
        ocT = oc.transpose(1, 0, 2).reshape(D, NQ).T  # [640, 2048]
        out[b, half * 512 : (half + 1) * 512] = ocT[0:512]
        state_next[b, half * 128 : (half + 1) * 128] = ocT[512:640]
    return out, state_next


if __name__ == "__main__":
    _build()
    print("build ok")
